# revision 1
# baseline (speedup 1.0000x reference)
"""Trainium2 Bass kernel for DeepNT-style GCN + path attention.

Problem (hardcoded shapes):
  GCN: h = relu(adj @ (x @ W0)); h = relu(adj @ (h @ W1)); emb = adj @ (h @ W2)
       adj [8192, 8192] f32, x [8192, 256], W0 [256,256], W1 [256,256], W2 [256,128]
  Attention: hu = emb[u], hv = emb[v], P = emb[paths]; 3 sequential residual
       scaled-dot-product refinements per side; out = cat(hu,hv) @ Wfc + bfc.

Distribution over 8 NeuronCores:
  - adj is row-sharded: core c owns output rows R_c = [c*1024, (c+1)*1024).
    The host supplies adj[R_c, :].T (k-major) so the PE can contract over k.
  - Each layer: every core computes the full T = H @ W contraction operand
    from all-gathered T-shards; core c computes its H-shard rows via
    out[:, R_c] = sum_k T[k, :] outer adjT[k, R_c] accumulated in PSUM.
    The H-shard is kept transposed in SBUF, projected by the next W, and the
    resulting T-shard is all-gathered (8.4 MB or 4.2 MB per layer).
  - Final emb shard is transposed back to natural row layout and all-gathered;
    u/v/path gathers + attention are data-parallel over the 4096 pairs
    (512 per core); the host concatenates the 8 output shards.
"""
import os
os.environ.setdefault("JAX_PLATFORMS", "")

import math
import numpy as np

import concourse.bacc as bacc
import concourse.tile as tile
import concourse.mybir as mybir
from concourse.bass import IndirectOffsetOnAxis
from concourse.bass_utils import run_bass_kernel_spmd
from concourse.masks import make_identity

NCORES = 8
N = 8192           # nodes
D_IN = 256
HID = 256
D_OUT = 128
B = 4096           # (u, v) pairs
NPATH = 3
PLEN = 10
SH = N // NCORES   # 1024 rows per core
BC = B // NCORES   # 512 pairs per core
SLOTS = BC // 128  # 4
PF = SLOTS * NPATH * PLEN  # 120 path rows gathered per partition

F32 = mybir.dt.float32
I32 = mybir.dt.int32
AX = mybir.AxisListType.X
MUL = mybir.AluOpType.mult
SUB = mybir.AluOpType.subtract
EXP = mybir.ActivationFunctionType.Exp


def _gcn_layer(nc, pools, t_full, adjT_view, NT, relu, ht_out):
    """One adj @ T contraction producing this core's H-shard, transposed.

    t_full:    DRAM [8, 1024, NT] all-gathered T (k-major, k = global node idx)
    adjT_view: DRAM AP [128, 64, 1024] = adjT (g, p) -> row g*128+p
    ht_out:    SBUF tile [128, NT//128, 1024] result (HT[:, R_c]), relu'd if relu
    """
    NH = NT // 128
    apool, tpool, psum_acc = pools
    acc = [[psum_acc.tile([128, 512], F32, name=f"acc_{nh}_{ih}", tag=f"acc_{nh}_{ih}")
            for ih in range(2)] for nh in range(NH)]
    a_tile = None
    for cr in range(NCORES):
        t_rank = tpool.tile([128, 8, NT], F32, tag="trank", name="t_rank")
        nc.sync.dma_start(
            t_rank[:], t_full[cr].rearrange("(kt p) n -> p kt n", p=128))
        for kt in range(8):
            ki = cr * 8 + kt
            if ki % 2 == 0:
                a_tile = apool.tile([128, 2, 1024], F32, tag="adj", name="a_tile")
                nc.sync.dma_start(a_tile[:], adjT_view[:, ki:ki + 2, :])
            for nh in range(NH):
                for ih in range(2):
                    nc.tensor.matmul(
                        acc[nh][ih][:],
                        lhsT=t_rank[:, kt, nh * 128:(nh + 1) * 128],
                        rhs=a_tile[:, ki % 2, ih * 512:(ih + 1) * 512],
                        start=(ki == 0), stop=(ki == 63))
    for nh in range(NH):
        for ih in range(2):
            dst = ht_out[:, nh, ih * 512:(ih + 1) * 512]
            if relu:
                nc.vector.tensor_scalar_max(dst, acc[nh][ih][:], 0.0)
            else:
                nc.vector.tensor_copy(dst, acc[nh][ih][:])


def _project_shard(nc, pools, ht_sb, w_sb, NT_out, t_out_sb):
    """T_next[R_c] = H[R_c] @ W from the transposed H-shard (lhsT = HT tiles)."""
    _, _, psum_small = pools
    for kt in range(8):
        ps = psum_small.tile([128, NT_out], F32, tag="tps", name="proj_ps")
        for dh in range(ht_sb.shape[1]):
            nc.tensor.matmul(
                ps[:], lhsT=ht_sb[:, dh, kt * 128:(kt + 1) * 128],
                rhs=w_sb[:, dh, :], start=(dh == 0),
                stop=(dh == ht_sb.shape[1] - 1))
        nc.vector.tensor_copy(t_out_sb[:, kt, :], ps[:])


def _allgather(nc, dram_pool, t_sb, NT, tag):
    """DMA the [128, 8, NT] shard to DRAM and AllGather to [8, 1024, NT]."""
    ag_in = dram_pool.tile([SH, NT], F32, name=f"agin_{tag}")
    nc.sync.dma_start(ag_in.rearrange("(kt p) n -> p kt n", p=128), t_sb[:])
    ag_out = dram_pool.tile([NCORES, SH, NT], F32, addr_space="Shared",
                            name=f"agout_{tag}")
    nc.gpsimd.collective_compute(
        "AllGather", mybir.AluOpType.bypass,
        replica_groups=[list(range(NCORES))],
        ins=[ag_in[:]], outs=[ag_out[:]])
    return ag_out


def _attention(nc, pools, q_sb, p_all, pp, identity, wq_sb, tag):
    """One residual scaled-dot-product refinement: q + softmax(P·(qWq)/sqrt(d))·P."""
    dpool, psum_d = pools
    k_sb = dpool.tile([128, SLOTS, 128], F32, tag="k_sb", name="k_sb")
    for slot in range(SLOTS):
        tp = psum_d.tile([128, 128], F32, tag="tp", name="att_tp")
        nc.tensor.transpose(tp[:], q_sb[:, slot, :], identity)
        qT = dpool.tile([128, 128], F32, tag="qT", name="qT")
        nc.vector.tensor_copy(qT[:], tp[:])
        kp = psum_d.tile([128, 128], F32, tag="kp", name="att_kp")
        nc.tensor.matmul(kp[:], lhsT=qT[:], rhs=wq_sb[:], start=True, stop=True)
        nc.vector.tensor_copy(k_sb[:, slot, :], kp[:])
    s_sb = dpool.tile([128, SLOTS, PLEN], F32, tag="s_sb", name="s_sb")
    for slot in range(SLOTS):
        psl = p_all[:, slot * NPATH * PLEN + pp * PLEN:
                    slot * NPATH * PLEN + (pp + 1) * PLEN, :]
        tmp = dpool.tile([128, PLEN, 128], F32, tag="tmp", name="att_tmp")
        nc.vector.tensor_tensor(
            tmp[:], psl, k_sb[:, slot, None, :].to_broadcast([128, PLEN, 128]),
            op=MUL)
        nc.vector.reduce_sum(s_sb[:, slot, :], tmp[:], axis=AX)
    mx = dpool.tile([128, SLOTS], F32, tag="mx", name="mx")
    nc.vector.reduce_max(mx[:], s_sb[:], axis=AX)
    e_sb = dpool.tile([128, SLOTS, PLEN], F32, tag="e_sb", name="e_sb")
    nc.vector.tensor_tensor(
        e_sb[:], s_sb[:], mx[:, :, None].to_broadcast([128, SLOTS, PLEN]), op=SUB)
    nc.scalar.activation(e_sb[:], e_sb[:], EXP, scale=float(1.0 / math.sqrt(D_OUT)))
    den = dpool.tile([128, SLOTS], F32, tag="mx", name="den")
    nc.vector.reduce_sum(den[:], e_sb[:], axis=AX)
    rden = dpool.tile([128, SLOTS], F32, tag="mx", name="rden")
    nc.vector.reciprocal(rden[:], den[:])
    nc.vector.tensor_tensor(
        e_sb[:], e_sb[:], rden[:, :, None].to_broadcast([128, SLOTS, PLEN]), op=MUL)
    q_new = dpool.tile([128, SLOTS, 128], F32, tag=f"q_{tag}", name="q_new")
    for slot in range(SLOTS):
        psl = p_all[:, slot * NPATH * PLEN + pp * PLEN:
                    slot * NPATH * PLEN + (pp + 1) * PLEN, :]
        tmp = dpool.tile([128, PLEN, 128], F32, tag="tmp", name="att_tmp2")
        nc.vector.tensor_tensor(
            tmp[:], psl, e_sb[:, slot, :, None].to_broadcast([128, PLEN, 128]),
            op=MUL)
        nc.vector.reduce_sum(q_new[:, slot, :],
                             tmp[:].rearrange("p l d -> p d l"), axis=AX)
    nc.vector.tensor_add(q_new[:], q_new[:], q_sb[:])
    return q_new


def build_program(repeats=1):
    """Build and compile the SPMD Bass program (identical on all 8 cores)."""
    nc = bacc.Bacc("TRN2", target_bir_lowering=False, debug=False,
                   num_devices=NCORES)
    adjT = nc.dram_tensor("adjT", [N, SH], F32, kind="ExternalInput")
    xT = nc.dram_tensor("xT", [D_IN, SH], F32, kind="ExternalInput")
    w0 = nc.dram_tensor("w0", [D_IN, HID], F32, kind="ExternalInput")
    w1 = nc.dram_tensor("w1", [HID, HID], F32, kind="ExternalInput")
    w2 = nc.dram_tensor("w2", [HID, D_OUT], F32, kind="ExternalInput")
    wq = nc.dram_tensor("wq", [D_OUT, D_OUT], F32, kind="ExternalInput")
    wu = nc.dram_tensor("wu", [128, D_OUT], F32, kind="ExternalInput")
    wv = nc.dram_tensor("wv", [128, D_OUT], F32, kind="ExternalInput")
    bfcb = nc.dram_tensor("bfcb", [128, 1], F32, kind="ExternalInput")
    u_idx = nc.dram_tensor("u_idx", [128, SLOTS], I32, kind="ExternalInput")
    v_idx = nc.dram_tensor("v_idx", [128, SLOTS], I32, kind="ExternalInput")
    p_idx = nc.dram_tensor("p_idx", [128, PF], I32, kind="ExternalInput")
    out = nc.dram_tensor("out", [BC], F32, kind="ExternalOutput")

    adjT_view = adjT.ap().rearrange("(g p) i -> p g i", p=128)

    with tile.TileContext(nc) as tc:
        with tc.tile_pool(name="const", bufs=1) as cpool:
            identity = cpool.tile([128, 128], F32, name="identity")
            make_identity(nc, identity[:])
            wq_sb = cpool.tile([128, D_OUT], F32, name="wq_sb")
            nc.sync.dma_start(wq_sb[:], wq.ap()[:])
            wu_sb = cpool.tile([128, D_OUT], F32, name="wu_sb")
            nc.sync.dma_start(wu_sb[:], wu.ap()[:])
            wv_sb = cpool.tile([128, D_OUT], F32, name="wv_sb")
            nc.sync.dma_start(wv_sb[:], wv.ap()[:])
            bfc_sb = cpool.tile([128, 1], F32, name="bfc_sb")
            nc.sync.dma_start(bfc_sb[:], bfcb.ap()[:])
            u_sb = cpool.tile([128, SLOTS], I32, name="u_sb")
            nc.sync.dma_start(u_sb[:], u_idx.ap()[:])
            v_sb = cpool.tile([128, SLOTS], I32, name="v_sb")
            nc.sync.dma_start(v_sb[:], v_idx.ap()[:])
            p_sb = cpool.tile([128, PF], I32, name="p_sb")
            nc.sync.dma_start(p_sb[:], p_idx.ap()[:])
            xT_sb = cpool.tile([128, 2, SH], F32, name="xT_sb")
            nc.sync.dma_start(xT_sb[:], xT.ap().rearrange("(dh p) k -> p dh k", p=128))
            w0_sb = cpool.tile([128, 2, HID], F32, name="w0_sb")
            nc.sync.dma_start(w0_sb[:], w0.ap().rearrange("(dh p) n -> p dh n", p=128))
            w1_sb = cpool.tile([128, 2, HID], F32, name="w1_sb")
            nc.sync.dma_start(w1_sb[:], w1.ap().rearrange("(dh p) n -> p dh n", p=128))
            w2_sb = cpool.tile([128, 2, D_OUT], F32, name="w2_sb")
            nc.sync.dma_start(w2_sb[:], w2.ap().rearrange("(dh p) n -> p dh n", p=128))
            # Sacrificial gather: the first indirect DMA of a program returns
            # corrupted data for partition 0 (cold descriptor ring); warm it up.
            warm = cpool.tile([128, SH], F32, name="warm")
            nc.gpsimd.indirect_dma_start(
                out=warm[:], out_offset=None, in_=adjT.ap()[:],
                in_offset=IndirectOffsetOnAxis(ap=u_sb[:, 0:1], axis=0))

            for _rep in range(repeats):
                _one_pass(nc, tc, adjT_view, identity, wq_sb, wu_sb, wv_sb,
                          bfc_sb, u_sb, v_sb, p_sb, xT_sb, w0_sb, w1_sb, w2_sb,
                          out)
    nc.compile()
    return nc


def _one_pass(nc, tc, adjT_view, identity, wq_sb, wu_sb, wv_sb, bfc_sb,
              u_sb, v_sb, p_sb, xT_sb, w0_sb, w1_sb, w2_sb, out):
    from contextlib import ExitStack
    with ExitStack() as ctx:
        dram = ctx.enter_context(tc.tile_pool(name="dram", bufs=1, space="DRAM"))
        hpool = ctx.enter_context(tc.tile_pool(name="hbuf", bufs=1))
        with ExitStack() as gctx:
            apool = gctx.enter_context(tc.tile_pool(name="adj_stream", bufs=4))
            tpool = gctx.enter_context(tc.tile_pool(name="t_stream", bufs=2))
            psum_acc = gctx.enter_context(
                tc.tile_pool(name="psum_acc", bufs=1, space="PSUM"))
            psum_small = gctx.enter_context(
                tc.tile_pool(name="psum_small", bufs=2, space="PSUM"))
            pools = (apool, tpool, psum_small)

            # T1 = x @ W0 shard, gathered
            t1_sb = hpool.tile([128, 8, HID], F32, name="t1_sb")
            _project_shard(nc, pools, xT_sb, w0_sb, HID, t1_sb)
            t1_full = _allgather(nc, dram, t1_sb, HID, "t1")

            gpools = (apool, tpool, psum_acc)
            h1_sb = hpool.tile([128, 2, SH], F32, name="h1_sb")
            _gcn_layer(nc, gpools, t1_full, adjT_view, HID, True, h1_sb)

            t2_sb = hpool.tile([128, 8, HID], F32, name="t2_sb")
            _project_shard(nc, pools, h1_sb, w1_sb, HID, t2_sb)
            t2_full = _allgather(nc, dram, t2_sb, HID, "t2")

            h2_sb = hpool.tile([128, 2, SH], F32, name="h2_sb")
            _gcn_layer(nc, gpools, t2_full, adjT_view, HID, True, h2_sb)

            t3_sb = hpool.tile([128, 8, D_OUT], F32, name="t3_sb")
            _project_shard(nc, pools, h2_sb, w2_sb, D_OUT, t3_sb)
            t3_full = _allgather(nc, dram, t3_sb, D_OUT, "t3")

            embT_sb = hpool.tile([128, 1, SH], F32, name="embT_sb")
            _gcn_layer(nc, gpools, t3_full, adjT_view, D_OUT, False, embT_sb)

            # transpose embT [d, i] -> emb natural rows [i, d], gather-ready
            emb_nat = hpool.tile([128, 8, D_OUT], F32, name="emb_nat")
            for it in range(8):
                tp = psum_small.tile([128, 128], F32, tag="ttp", name="emb_tp")
                nc.tensor.transpose(
                    tp[:], embT_sb[:, 0, it * 128:(it + 1) * 128], identity[:])
                nc.vector.tensor_copy(emb_nat[:, it, :], tp[:])
            emb_full = _allgather(nc, dram, emb_nat, D_OUT, "emb")

        # ---- phase 2: gathers + attention, data-parallel over pairs ----
        emb_table = emb_full.rearrange("c r d -> (c r) d")
        with ExitStack() as dctx:
            dpool = dctx.enter_context(tc.tile_pool(name="attn", bufs=2))
            ppool = dctx.enter_context(tc.tile_pool(name="pgather", bufs=1))
            psum_d = dctx.enter_context(
                tc.tile_pool(name="psum_d", bufs=2, space="PSUM"))
            hu = dpool.tile([128, SLOTS, D_OUT], F32, tag="q_u", name="hu")
            hv = dpool.tile([128, SLOTS, D_OUT], F32, tag="q_v", name="hv")
            for slot in range(SLOTS):
                nc.gpsimd.indirect_dma_start(
                    out=hu[:, slot, :], out_offset=None, in_=emb_table,
                    in_offset=IndirectOffsetOnAxis(ap=u_sb[:, slot:slot + 1],
                                                   axis=0))
                nc.gpsimd.indirect_dma_start(
                    out=hv[:, slot, :], out_offset=None, in_=emb_table,
                    in_offset=IndirectOffsetOnAxis(ap=v_sb[:, slot:slot + 1],
                                                   axis=0))
            p_all = ppool.tile([128, PF, D_OUT], F32, name="p_all")
            for f in range(PF):
                nc.gpsimd.indirect_dma_start(
                    out=p_all[:, f, :], out_offset=None, in_=emb_table,
                    in_offset=IndirectOffsetOnAxis(ap=p_sb[:, f:f + 1], axis=0))

            atp = (dpool, psum_d)
            for pp in range(NPATH):
                hu = _attention(nc, atp, hu, p_all[:], pp, identity[:], wq_sb,
                                "u")
                hv = _attention(nc, atp, hv, p_all[:], pp, identity[:], wq_sb,
                                "v")

            pu = dpool.tile([128, SLOTS, D_OUT], F32, tag="tmp", name="pu")
            nc.vector.tensor_tensor(
                pu[:], hu[:], wu_sb[:, None, :].to_broadcast([128, SLOTS, D_OUT]),
                op=MUL)
            fu = dpool.tile([128, SLOTS], F32, tag="fu", name="fu")
            nc.vector.reduce_sum(fu[:], pu[:], axis=AX)
            pv = dpool.tile([128, SLOTS, D_OUT], F32, tag="tmp", name="pv")
            nc.vector.tensor_tensor(
                pv[:], hv[:], wv_sb[:, None, :].to_broadcast([128, SLOTS, D_OUT]),
                op=MUL)
            fv = dpool.tile([128, SLOTS], F32, tag="fv", name="fv")
            nc.vector.reduce_sum(fv[:], pv[:], axis=AX)
            osb = dpool.tile([128, SLOTS], F32, tag="osb", name="osb")
            nc.vector.tensor_add(osb[:], fu[:], fv[:])
            nc.vector.tensor_scalar_add(osb[:], osb[:], bfc_sb[:])
            nc.sync.dma_start(out.ap().rearrange("(s p) -> p s", p=128), osb[:])


_PROGRAM_CACHE = {}


def _get_program(repeats=1):
    if repeats not in _PROGRAM_CACHE:
        _PROGRAM_CACHE[repeats] = build_program(repeats)
    return _PROGRAM_CACHE[repeats]


def make_in_maps(x, u, v, adj, paths, W0, W1, W2, Wq, Wfc, bfc):
    """Shard + lay out the full inputs for the 8 cores."""
    x = np.asarray(x, np.float32)
    adj = np.asarray(adj, np.float32)
    u = np.asarray(u).astype(np.int32)
    v = np.asarray(v).astype(np.int32)
    paths = np.asarray(paths).astype(np.int32)
    W0 = np.asarray(W0, np.float32)
    W1 = np.asarray(W1, np.float32)
    W2 = np.asarray(W2, np.float32)
    Wq = np.asarray(Wq, np.float32)
    Wfc = np.asarray(Wfc, np.float32).reshape(2 * D_OUT)
    bfc = np.asarray(bfc, np.float32).reshape(1)

    adjT_all = np.ascontiguousarray(adj.T)          # [N, N]: adjT[k, i]
    xT_all = np.ascontiguousarray(x.T)              # [D_IN, N]
    wu = np.ascontiguousarray(
        np.broadcast_to(Wfc[:D_OUT][None, :], (128, D_OUT)))
    wv = np.ascontiguousarray(
        np.broadcast_to(Wfc[D_OUT:][None, :], (128, D_OUT)))
    bfcb = np.full((128, 1), bfc[0], np.float32)

    in_maps = []
    for c in range(NCORES):
        rows = slice(c * SH, (c + 1) * SH)
        bs = slice(c * BC, (c + 1) * BC)
        # batch-shard index layouts: b_loc = slot*128 + p  ->  [p, slot(, ...)]
        u_c = np.ascontiguousarray(u[bs].reshape(SLOTS, 128).T)
        v_c = np.ascontiguousarray(v[bs].reshape(SLOTS, 128).T)
        p_c = np.ascontiguousarray(
            paths[bs].reshape(SLOTS, 128, NPATH, PLEN)
            .transpose(1, 0, 2, 3).reshape(128, PF))
        in_maps.append({
            "adjT": np.ascontiguousarray(adjT_all[:, rows]),
            "xT": np.ascontiguousarray(xT_all[:, rows]),
            "w0": W0, "w1": W1, "w2": W2, "wq": Wq,
            "wu": wu, "wv": wv, "bfcb": bfcb,
            "u_idx": u_c, "v_idx": v_c, "p_idx": p_c,
        })
    return in_maps


def kernel(x, u, v, adj, paths, W0, W1, W2, Wq, Wfc, bfc):
    """Full-input entry point: shards across 8 cores, runs, reassembles."""
    nc = _get_program(repeats=1)
    in_maps = make_in_maps(x, u, v, adj, paths, W0, W1, W2, Wq, Wfc, bfc)
    res = run_bass_kernel_spmd(nc, in_maps, core_ids=list(range(NCORES)))
    return np.concatenate([res.results[c]["out"] for c in range(NCORES)], axis=0)
